# Initial kernel scaffold
#
"""Trainium2 Bass kernel for nn_DecoderBlock (T=S=1024, B=4, E=1024, H=16,
DH=64, DFF=4096) on 8 NeuronCores.

Sharding: core = b*2 + th  (b in 0..3 batches, th in {0,1} query halves).
Each core computes the full decoder block for its 512 query rows of its
batch; no collectives. The SPMD program is identical on all cores; all
per-core differences (which batch, which query half, attention masks) are
carried by the input data prepared host-side.

Mask exploitation is data-driven: the host inspects the actual masks,
derives the number of live 128-wide key blocks (kv extents), and ships 0/1
multiplicative mask tiles only where a block is partially masked. With the
reference masks (causal + last-quarter key padding) this skips 25% of K/V
work and the fully-masked score blocks beyond the kv extent.

Matmuls run in bf16 with fp32 PSUM accumulation; softmax/normalization/
layernorm statistics run in fp32.
"""
import sys

sys.path.insert(0, "/opt/trn_rl_repo")

import numpy as np
import ml_dtypes

import concourse.bass as bass
from concourse import bacc
import concourse.mybir as mybir
import concourse.tile as tile
from concourse.bass_utils import run_bass_kernel_spmd
from concourse.masks import make_identity

F32 = mybir.dt.float32
BF16 = mybir.dt.bfloat16
AF = mybir.ActivationFunctionType
ALU = mybir.AluOpType
BF16NP = ml_dtypes.bfloat16

T = 1024
B = 4
E = 1024
H = 16
DH = 64
DFF = 4096
P = 128
TLOC = T // 2          # query rows per core
NTB = TLOC // P        # 4 query-row blocks per core
NEC = E // P           # 8 feature chunks
NFC = DFF // P         # 32 ffn chunks

_PROGRAM_CACHE = {}


def _attention(nc, po, *, yqT, kvT, kbs, wq, wk, wv, wp, bp_sb,
               mask_T=None, resid, out_pre, tag):
    """One multi-head attention + projection + residual.

    yqT:  SBUF tile [128, NEC, TLOC] bf16 — query-side activations, T-layout
    kvT:  SBUF tile [128, NEC, kbs*128] bf16 — key/value side, T-layout
    wq/wk/wv/wp: DRAM [E, E] bf16 (head-major cols; wq pre-scaled 1/sqrt(dh))
    mask_T: SBUF tile [128, kbs, TLOC] bf16 0/1 keep-mask or None
    resid/out_pre: [128, NTB, E] f32 R-layout tiles
    """
    SKV = kbs * P
    nsh = (SKV + 511) // 512

    # ---- Q^T and K^T per head-pair ----
    qT = po["cols"].tile([P, NEC, TLOC], BF16, tag="colsBF", name=f"{tag}_qT")
    kT = po["attn"].tile([P, NEC, SKV], BF16, tag="kvT", name=f"{tag}_kT")
    for p in range(8):
        wq_t = po["w"].tile([P, NEC, P], BF16, tag="w_qk", bufs=3, name=f"{tag}_wq{p}")
        nc.sync.dma_start(
            wq_t[:],
            wq[:, p * P:(p + 1) * P].rearrange("(eo pp) c -> pp eo c", pp=P))
        ps = po["psA"].tile([P, 512], F32, tag="sps", name=f"{tag}_psq{p}")
        for ec in range(NEC):
            nc.tensor.matmul(ps[:, :TLOC], wq_t[:, ec, :], yqT[:, ec, :],
                             start=(ec == 0), stop=(ec == NEC - 1))
        nc.vector.tensor_copy(qT[:, p, :], ps[:, :TLOC])

        wk_t = po["w"].tile([P, NEC, P], BF16, tag="w_qk", bufs=3, name=f"{tag}_wk{p}")
        nc.sync.dma_start(
            wk_t[:],
            wk[:, p * P:(p + 1) * P].rearrange("(eo pp) c -> pp eo c", pp=P))
        for sh in range(nsh):
            w = min(512, SKV - sh * 512)
            ps = po["psA"].tile([P, 512], F32, tag="sps",
                                name=f"{tag}_psk{p}_{sh}")
            for ec in range(NEC):
                nc.tensor.matmul(ps[:, :w], wk_t[:, ec, :],
                                 kvT[:, ec, sh * 512: sh * 512 + w],
                                 start=(ec == 0), stop=(ec == NEC - 1))
            nc.vector.tensor_copy(kT[:, p, sh * 512: sh * 512 + w], ps[:, :w])

    # ---- V in R-layout [s-chunk partitions, head cols] ----
    vR = po["attn"].tile([P, kbs, E], BF16, tag="vR", bufs=1,
                         name=f"{tag}_vR")
    for eh in range(2):
        wv_t = po["w"].tile([P, NEC, 512], BF16, tag="w_v", bufs=3,
                            name=f"{tag}_wv{eh}")
        nc.sync.dma_start(
            wv_t[:], wv[:, eh * 512:(eh + 1) * 512].rearrange(
                "(eo pp) c -> pp eo c", pp=P))
        for sc in range(kbs):
            ps = po["psA"].tile([P, 512], F32, tag="sps",
                                name=f"{tag}_psv{eh}_{sc}")
            for ec in range(NEC):
                nc.tensor.matmul(ps[:], kvT[:, ec, sc * P:(sc + 1) * P],
                                 wv_t[:, ec, :],
                                 start=(ec == 0), stop=(ec == NEC - 1))
            nc.vector.tensor_copy(vR[:, sc, eh * 512:(eh + 1) * 512], ps[:])

    # ---- attention per group of 4 heads (2 pairs) ----
    # Emission order targets HW concurrency: score matmuls for a head pair
    # are adjacent (row groups 0/64 run concurrently), attnV matmuls for the
    # 4 heads are adjacent (col groups), denominator matmuls are contiguous
    # (shared ones lhsT loaded once, 4-way col-group concurrency).
    oT = po["cols"].tile([P, NEC, TLOC], BF16, tag="colsBF", name=f"{tag}_oT")
    for g in range(4):
        dps = po["psB"].tile([P, 512], F32, tag="dps", name=f"{tag}_dps{g}")
        zts = {}
        pTs = {}
        for p in (2 * g, 2 * g + 1):
            for half in range(2):
                h = 2 * p + half
                pTs[h] = po["attn"].tile([P, kbs, TLOC], BF16, tag="pT",
                                         bufs=5, name=f"{tag}_pT{h}")
            for kb in range(kbs):
                sps2 = []
                for half in range(2):
                    hb = 64 * half
                    sps = po["psA"].tile([P, 512], F32, tag="sps",
                                         name=f"{tag}_sps{p}_{kb}_{half}")
                    nc.tensor.matmul(
                        sps[:, :TLOC],
                        kT[hb:hb + 64, p, kb * P:(kb + 1) * P],
                        qT[hb:hb + 64, p, :],
                        start=True, stop=True, tile_position=(hb, 0))
                    sps2.append(sps)
                for half in range(2):
                    h = 2 * p + half
                    nc.scalar.activation(pTs[h][:, kb, :],
                                         sps2[half][:, :TLOC], AF.Exp)
                    if mask_T is not None:
                        nc.vector.tensor_tensor(
                            pTs[h][:, kb, :], pTs[h][:, kb, :],
                            mask_T[:, kb, :], ALU.mult)
        for p in (2 * g, 2 * g + 1):
            zts[p] = po["psB"].tile([P, 512], F32, tag="zt",
                                    name=f"{tag}_zt{p}")
        for kb in range(kbs):
            for p in (2 * g, 2 * g + 1):
                for half in range(2):
                    h = 2 * p + half
                    hb = 64 * half
                    nc.tensor.matmul(
                        zts[p][hb:hb + 64, :TLOC],
                        vR[:, kb, h * 64:(h + 1) * 64],
                        pTs[h][:, kb, :],
                        start=(kb == 0), stop=(kb == kbs - 1),
                        tile_position=(0, hb))
        for kb in range(kbs):
            for gh in range(4):
                h = 4 * g + gh
                nc.tensor.matmul(
                    dps[32 * gh:32 * gh + 1, :TLOC],
                    po["ones_bf"][:, 0:1], pTs[h][:, kb, :],
                    start=(kb == 0), stop=(kb == kbs - 1),
                    tile_position=(0, 32 * gh))
        # denominators -> reciprocals
        dsb = po["scr"].tile([P, 512], F32, tag="dsb", name=f"{tag}_dsb{g}")
        nc.vector.memset(dsb[:], 1.0)
        for gh in range(4):
            nc.scalar.copy(dsb[32 * gh:32 * gh + 1, :],
                           dps[32 * gh:32 * gh + 1, :])
        rsb = po["scr"].tile([P, 512], F32, tag="rsb", name=f"{tag}_rsb{g}")
        nc.vector.reciprocal(out=rsb[:], in_=dsb[:])
        # broadcast recips and normalize
        for p in (2 * g, 2 * g + 1):
            bcp = po["psA"].tile([P, 512], F32, tag="sps",
                                 name=f"{tag}_bcp{p}")
            for half in range(2):
                h = 2 * p + half
                gh = h % 4
                hb = 64 * half
                nc.tensor.matmul(
                    bcp[hb:hb + 64, :TLOC],
                    po["ones_f32"][32 * gh:32 * gh + 1, 0:64],
                    rsb[32 * gh:32 * gh + 1, :TLOC],
                    start=True, stop=True,
                    tile_position=(32 * gh, hb))
            bcs = po["scr"].tile([P, 512], F32, tag="bcs", name=f"{tag}_bc{p}")
            nc.scalar.copy(bcs[:, :TLOC], bcp[:, :TLOC])
            nc.vector.tensor_tensor(oT[:, p, :], zts[p][:, :TLOC],
                                    bcs[:, :TLOC], ALU.mult)

    # ---- projection + bias + residual ----
    for tb in range(NTB):
        for eh in range(2):
            ps = po["psA"].tile([P, 512], F32, tag="sps",
                                name=f"{tag}_pspr{tb}_{eh}")
            for p in range(8):
                wp_t = po["w"].tile([P, 512], BF16, tag="w_p", bufs=4,
                                    name=f"{tag}_wp{tb}_{eh}_{p}")
                nc.sync.dma_start(
                    wp_t[:],
                    wp[p * P:(p + 1) * P, eh * 512:(eh + 1) * 512])
                nc.tensor.matmul(ps[:], oT[:, p, tb * P:(tb + 1) * P],
                                 wp_t[:], start=(p == 0), stop=False)
            nc.tensor.matmul(ps[:], po["ones_bf"][0:1, 0:P],
                             bp_sb[0:1, eh * 512:(eh + 1) * 512],
                             start=False, stop=True)
            nc.vector.tensor_tensor(
                out_pre[:, tb, eh * 512:(eh + 1) * 512], ps[:],
                resid[:, tb, eh * 512:(eh + 1) * 512], ALU.add)


def _ln_and_transpose(nc, po, *, src, outR, dst_T=None, gb=None, tag=""):
    """Per-row layernorm of [128, NTB, E] f32 + optional bf16 transpose to
    T-layout [128, NEC, TLOC]."""
    for tb in range(NTB):
        stats = po["scr"].tile([P, 2, 6], F32, tag="ln_st",
                               name=f"{tag}_st{tb}")
        nc.vector.bn_stats(stats[:, 0, :], src[:, tb, 0:512])
        nc.vector.bn_stats(stats[:, 1, :], src[:, tb, 512:1024])
        mv = po["scr"].tile([P, 2], F32, tag="ln_mv", name=f"{tag}_mv{tb}")
        nc.vector.bn_aggr(mv[:], stats[:])
        nmean = po["scr"].tile([P, 1], F32, tag="ln_nm", name=f"{tag}_nm{tb}")
        nc.vector.tensor_scalar_mul(nmean[:], mv[:, 0:1], -1.0)
        rstd = po["scr"].tile([P, 1], F32, tag="ln_rs", name=f"{tag}_rs{tb}")
        nc.scalar.activation(rstd[:], mv[:, 1:2], AF.Sqrt,
                             bias=po["eps"][:])
        nc.vector.reciprocal(out=rstd[:], in_=rstd[:])
        nc.vector.tensor_scalar(outR[:, tb, :], src[:, tb, :],
                                nmean[:], rstd[:], ALU.add, ALU.mult)
        if gb is not None:
            g_bc, b_bc = gb
            nc.vector.tensor_tensor(outR[:, tb, :], outR[:, tb, :],
                                    g_bc[:], ALU.mult)
            nc.vector.tensor_tensor(outR[:, tb, :], outR[:, tb, :],
                                    b_bc[:], ALU.add)
        if dst_T is not None:
            ybf = po["scr"].tile([P, E], BF16, tag="ybf", name=f"{tag}_yb{tb}")
            nc.vector.tensor_copy(ybf[:], outR[:, tb, :])
            for eg in range(2):
                pt = po["psA"].tile([P, 4, P], BF16, tag="sps",
                                    name=f"{tag}_tr{tb}_{eg}")
                for j in range(4):
                    ec = eg * 4 + j
                    nc.tensor.transpose(pt[:, j, :],
                                        ybf[:, ec * P:(ec + 1) * P],
                                        po["ident"][:])
                for j in range(4):
                    ec = eg * 4 + j
                    nc.scalar.copy(dst_T[:, ec, tb * P:(tb + 1) * P],
                                   pt[:, j, :])


def _broadcast_row(nc, po, src_row, width, tag):
    """Broadcast [1, width] f32 SBUF row (base 0) -> [128, width] f32."""
    out = po["persist"].tile([P, E], F32, tag=tag, name=tag)
    for c in range(0, width, 512):
        w = min(512, width - c)
        ps = po["psA"].tile([P, 512], F32, tag="sps", name=f"{tag}_bc{c}")
        nc.tensor.matmul(ps[0:P, :w], po["ones_f32"][0:1, 0:P],
                         src_row[0:1, c:c + w], start=True, stop=True)
        nc.scalar.copy(out[:, c:c + w], ps[:, :w])
    return out


def build_program(kbs_s, kbs_c, use_mask_s, use_mask_c, use_gb):
    nc = bacc.Bacc("TRN2", target_bir_lowering=False, debug=False,
                   num_devices=8)
    SKV_S = kbs_s * P
    SKV_C = kbs_c * P

    def di(name, shape, dt=BF16):
        return nc.dram_tensor(name, shape, dt, kind="ExternalInput")

    xTq = di("xTq", [E, TLOC])
    xTkv = di("xTkv", [E, SKV_S])
    xres = di("xres", [TLOC, E], F32)
    yencT = di("yencT", [E, SKV_C])
    wq1 = di("wq1", [E, E]); wk1 = di("wk1", [E, E]); wv1 = di("wv1", [E, E])
    wp1 = di("wp1", [E, E]); bp1 = di("bp1", [1, E])
    wq2 = di("wq2", [E, E]); wk2 = di("wk2", [E, E]); wv2 = di("wv2", [E, E])
    wp2 = di("wp2", [E, E]); bp2 = di("bp2", [1, E])
    w1 = di("w1", [E, DFF]); b1c = di("b1c", [P, NFC], F32)
    w2 = di("w2", [DFF, E]); b2 = di("b2", [1, E])
    if use_mask_s:
        mask_s = di("mask_s", [SKV_S, TLOC])
    if use_mask_c:
        mask_c = di("mask_c", [SKV_C, TLOC])
    if use_gb:
        lngb = di("lngb", [1, 6 * E], F32)
    out = nc.dram_tensor("out", [TLOC, E], F32, kind="ExternalOutput")

    with tile.TileContext(nc) as tc:
        with (
            tc.tile_pool(name="persist", bufs=1) as persist,
            tc.tile_pool(name="rows", bufs=2) as rows,
            tc.tile_pool(name="cols", bufs=3) as cols,
            tc.tile_pool(name="wpool", bufs=2) as wpool,
            tc.tile_pool(name="scr", bufs=2) as scr,
            tc.tile_pool(name="psA", bufs=4, space="PSUM") as psA,
        ):
            po = dict(persist=persist, rows=rows, cols=cols, w=wpool,
                      scr=scr, psA=psA)

            ones_bf = persist.tile([P, P], BF16)
            nc.vector.memset(ones_bf[:], 1.0)
            ones_f32 = persist.tile([P, 64], F32)
            nc.vector.memset(ones_f32[:], 1.0)
            ident = persist.tile([P, P], BF16)
            make_identity(nc, ident[:])
            eps_tile = persist.tile([P, 1], F32)
            nc.vector.memset(eps_tile[:], 1e-5)
            po.update(ones_bf=ones_bf, ones_f32=ones_f32, ident=ident,
                      eps=eps_tile)

            bp1_sb = persist.tile([1, E], BF16, tag="bp1", name="bp1s")
            nc.sync.dma_start(bp1_sb[:], bp1[:])
            bp2_sb = persist.tile([1, E], BF16, tag="bp2", name="bp2s")
            nc.sync.dma_start(bp2_sb[:], bp2[:])
            b2_sb = persist.tile([1, E], BF16, tag="b2", name="b2s")
            nc.sync.dma_start(b2_sb[:], b2[:])
            b1_sb = persist.tile([P, NFC], F32, tag="b1c", name="b1s")
            nc.sync.dma_start(b1_sb[:], b1c[:])

            gbs = [None, None, None]
            if use_gb:
                gbrow = persist.tile([1, 6 * E], F32, tag="lngb", name="gbr")
                nc.sync.dma_start(gbrow[:], lngb[:])
                for i in range(3):
                    g_bc = _broadcast_row(
                        nc, po, gbrow[:, 2 * i * E:(2 * i + 1) * E], E,
                        f"g_bc{i}")
                    b_bc = _broadcast_row(
                        nc, po, gbrow[:, (2 * i + 1) * E:(2 * i + 2) * E], E,
                        f"b_bc{i}")
                    gbs[i] = (g_bc, b_bc)

            xres_sb = rows.tile([P, NTB, E], F32, tag="rowsF32", name="xresS")
            for tb in range(NTB):
                nc.sync.dma_start(xres_sb[:, tb, :],
                                  xres[tb * P:(tb + 1) * P, :])
            y1pre = rows.tile([P, NTB, E], F32, tag="rowsF32", name="y1pre")
            y1R = rows.tile([P, NTB, E], F32, tag="rowsF32", name="y1R")
            y1T = cols.tile([P, NEC, TLOC], BF16, tag="colsBF", name="y1T")
            y2pre = rows.tile([P, NTB, E], F32, tag="rowsF32", name="y2pre")
            y2R = rows.tile([P, NTB, E], F32, tag="rowsF32", name="y2R")
            y2T = cols.tile([P, NEC, TLOC], BF16, tag="colsBF", name="y2T")

            with (
                tc.tile_pool(name="attn_sb", bufs=2) as attn_sb,
                tc.tile_pool(name="psB", bufs=2, space="PSUM") as psB,
            ):
                po["attn"] = attn_sb
                po["psB"] = psB

                mask_s_sb = None
                if use_mask_s:
                    mask_s_sb = attn_sb.tile([P, kbs_s, TLOC], BF16,
                                             tag="mask_s", bufs=1,
                                             name="mask_sS")
                    nc.sync.dma_start(
                        mask_s_sb[:],
                        mask_s.rearrange("(kb p) t -> p kb t", p=P))
                mask_c_sb = None
                if use_mask_c:
                    mask_c_sb = attn_sb.tile([P, kbs_c, TLOC], BF16,
                                             tag="mask_c", bufs=1,
                                             name="mask_cS")
                    nc.sync.dma_start(
                        mask_c_sb[:],
                        mask_c.rearrange("(kb p) t -> p kb t", p=P))

                xTq_sb = cols.tile([P, NEC, TLOC], BF16, tag="colsBF",
                                   name="xTqS")
                for ec in range(NEC):
                    nc.sync.dma_start(
                        xTq_sb[:, ec, :], xTq[ec * P:(ec + 1) * P, :])
                xTkv_sb = attn_sb.tile([P, NEC, SKV_S], BF16, tag="kvT",
                                       name="xTkvS")
                for ec in range(NEC):
                    nc.sync.dma_start(
                        xTkv_sb[:, ec, :], xTkv[ec * P:(ec + 1) * P, :])

                _attention(nc, po, yqT=xTq_sb, kvT=xTkv_sb, kbs=kbs_s,
                           wq=wq1, wk=wk1, wv=wv1, wp=wp1, bp_sb=bp1_sb,
                           mask_T=mask_s_sb, resid=xres_sb, out_pre=y1pre,
                           tag="sa")
                _ln_and_transpose(nc, po, src=y1pre, outR=y1R, dst_T=y1T,
                                  gb=gbs[0], tag="ln1")

                yencT_sb = attn_sb.tile([P, NEC, SKV_C], BF16, tag="kvT",
                                        name="yencTS")
                for ec in range(NEC):
                    nc.sync.dma_start(
                        yencT_sb[:, ec, :], yencT[ec * P:(ec + 1) * P, :])
                _attention(nc, po, yqT=y1T, kvT=yencT_sb, kbs=kbs_c,
                           wq=wq2, wk=wk2, wv=wv2, wp=wp2, bp_sb=bp2_sb,
                           mask_T=mask_c_sb, resid=y1R, out_pre=y2pre,
                           tag="ca")
                _ln_and_transpose(nc, po, src=y2pre, outR=y2R, dst_T=y2T,
                                  gb=gbs[1], tag="ln2")

            with (
                tc.tile_pool(name="ffn_sb", bufs=1) as ffn_sb,
                tc.tile_pool(name="psC", bufs=4, space="PSUM") as psC,
            ):
                hT = ffn_sb.tile([P, NFC, TLOC], BF16, tag="hT", name="hT")
                for fc in range(NFC):
                    w1_t = wpool.tile([P, NEC, P], BF16, tag="w_f1", bufs=3,
                                      name=f"w1_{fc}")
                    nc.sync.dma_start(
                        w1_t[:],
                        w1[:, fc * P:(fc + 1) * P].rearrange(
                            "(eo pp) c -> pp eo c", pp=P))
                    ps = psA.tile([P, 512], F32, tag="sps", name=f"psf1_{fc}")
                    for ec in range(NEC):
                        nc.tensor.matmul(ps[:, :TLOC], w1_t[:, ec, :],
                                         y2T[:, ec, :],
                                         start=(ec == 0),
                                         stop=(ec == NEC - 1))
                    nc.scalar.activation(hT[:, fc, :], ps[:, :TLOC], AF.Relu,
                                         bias=b1_sb[:, fc:fc + 1])

                y3pre = rows.tile([P, NTB, E], F32, tag="rowsF32",
                                  name="y3pre")
                for eh in range(2):
                    pss = [psC.tile([P, 512], F32, tag="ps_f2",
                                    name=f"psf2_{eh}_{tb}")
                           for tb in range(NTB)]
                    for fc in range(NFC):
                        w2_t = wpool.tile([P, 512], BF16, tag="w_f2", bufs=4,
                                          name=f"w2_{eh}_{fc}")
                        nc.sync.dma_start(
                            w2_t[:], w2[fc * P:(fc + 1) * P,
                                        eh * 512:(eh + 1) * 512])
                        for tb in range(NTB):
                            nc.tensor.matmul(
                                pss[tb][:], hT[:, fc, tb * P:(tb + 1) * P],
                                w2_t[:], start=(fc == 0), stop=False)
                    for tb in range(NTB):
                        nc.tensor.matmul(pss[tb][:], ones_bf[0:1, 0:P],
                                         b2_sb[0:1, eh * 512:(eh + 1) * 512],
                                         start=False, stop=True)
                        nc.vector.tensor_tensor(
                            y3pre[:, tb, eh * 512:(eh + 1) * 512],
                            pss[tb][:],
                            y2R[:, tb, eh * 512:(eh + 1) * 512], ALU.add)

                outR = rows.tile([P, NTB, E], F32, tag="rowsF32", name="outR")
                _ln_and_transpose(nc, po, src=y3pre, outR=outR, gb=gbs[2],
                                  tag="ln3")
                nc.sync.dma_start(out.rearrange("(tb p) e -> p tb e", p=P),
                                  outR[:])

    nc.compile()
    return nc


def _prep_inputs(inputs):
    """Host-side prep: returns (program_key, 8 in_maps, host_ln3)."""
    tgt = np.asarray(inputs["tgt"], np.float32)
    yenc = np.asarray(inputs["Y_enc_out"], np.float32)
    tgt_mask = np.asarray(inputs["tgt_mask"], np.float32)
    spm = np.asarray(inputs["src_padding_mask"])
    tpm = np.asarray(inputs["tgt_padding_mask"])

    causal = np.isneginf(tgt_mask) | np.isnan(tgt_mask)   # [Tq, Sk]
    masked_s = causal[None, :, :] | tpm[:, None, :]       # [B, Tq, Sk]
    masked_c = np.zeros((B, T, T), bool) | spm[:, None, :]

    live_s = ~masked_s.all(axis=(0, 1))
    live_c = ~masked_c.all(axis=(0, 1))
    kbs_s = max(1, -(-int(np.max(np.nonzero(live_s)[0], initial=0) + 1) // P))
    kbs_c = max(1, -(-int(np.max(np.nonzero(live_c)[0], initial=0) + 1) // P))

    keep_s = (~masked_s[:, :, :kbs_s * P]).astype(np.float32)
    keep_c = (~masked_c[:, :, :kbs_c * P]).astype(np.float32)
    use_mask_s = not np.all(keep_s == 1.0)
    use_mask_c = not np.all(keep_c == 1.0)

    g1 = np.asarray(inputs["ln1_g"], np.float32)
    b1g = np.asarray(inputs["ln1_b"], np.float32)
    g2 = np.asarray(inputs["ln2_g"], np.float32)
    b2g = np.asarray(inputs["ln2_b"], np.float32)
    g3 = np.asarray(inputs["ln3_g"], np.float32)
    b3g = np.asarray(inputs["ln3_b"], np.float32)
    use_gb = not (np.all(g1 == 1) and np.all(g2 == 1) and np.all(b1g == 0)
                  and np.all(b2g == 0))
    host_ln3 = None
    if not (np.all(g3 == 1) and np.all(b3g == 0)):
        host_ln3 = (g3, b3g)

    def heads_cols(w):  # [H, E, DH] -> [E, H*DH]
        return np.ascontiguousarray(
            np.asarray(w, np.float32).transpose(1, 0, 2).reshape(E, E))

    scale = 1.0 / np.sqrt(np.float32(DH))
    wq1 = (heads_cols(inputs["Wq1"]) * scale).astype(BF16NP)
    wk1 = heads_cols(inputs["Wk1"]).astype(BF16NP)
    wv1 = heads_cols(inputs["Wv1"]).astype(BF16NP)
    wq2 = (heads_cols(inputs["Wq2"]) * scale).astype(BF16NP)
    wk2 = heads_cols(inputs["Wk2"]).astype(BF16NP)
    wv2 = heads_cols(inputs["Wv2"]).astype(BF16NP)
    wp1 = np.asarray(inputs["Wp1"], np.float32).astype(BF16NP)
    wp2 = np.asarray(inputs["Wp2"], np.float32).astype(BF16NP)
    w1 = np.asarray(inputs["W1"], np.float32).astype(BF16NP)
    w2 = np.asarray(inputs["W2"], np.float32).astype(BF16NP)
    bp1 = np.asarray(inputs["bp1"], np.float32).reshape(1, E).astype(BF16NP)
    bp2 = np.asarray(inputs["bp2"], np.float32).reshape(1, E).astype(BF16NP)
    b2v = np.asarray(inputs["b2"], np.float32).reshape(1, E).astype(BF16NP)
    b1c = np.ascontiguousarray(
        np.asarray(inputs["b1"], np.float32).reshape(NFC, P).T)
    lngb = np.concatenate([g1, b1g, g2, b2g, g3, b3g]).reshape(1, 6 * E)

    in_maps = []
    for core in range(8):
        b = core // 2
        th = core % 2
        t0 = th * TLOC
        xb = tgt[:, b, :]
        xT = np.ascontiguousarray(xb.T)
        m = {
            "xTq": np.ascontiguousarray(xT[:, t0:t0 + TLOC]).astype(BF16NP),
            "xTkv": np.ascontiguousarray(xT[:, :kbs_s * P]).astype(BF16NP),
            "xres": np.ascontiguousarray(xb[t0:t0 + TLOC, :]),
            "yencT": np.ascontiguousarray(
                yenc[:kbs_c * P, b, :].T).astype(BF16NP),
            "wq1": wq1, "wk1": wk1, "wv1": wv1, "wp1": wp1, "bp1": bp1,
            "wq2": wq2, "wk2": wk2, "wv2": wv2, "wp2": wp2, "bp2": bp2,
            "w1": w1, "b1c": b1c, "w2": w2, "b2": b2v,
        }
        if use_mask_s:
            m["mask_s"] = np.ascontiguousarray(
                keep_s[b, t0:t0 + TLOC, :].T).astype(BF16NP)
        if use_mask_c:
            m["mask_c"] = np.ascontiguousarray(
                keep_c[b, t0:t0 + TLOC, :].T).astype(BF16NP)
        if use_gb:
            m["lngb"] = lngb
        in_maps.append(m)

    key = (kbs_s, kbs_c, use_mask_s, use_mask_c, use_gb)
    return key, in_maps, host_ln3


def kernel(**inputs) -> np.ndarray:
    key, in_maps, host_ln3 = _prep_inputs(inputs)
    if key not in _PROGRAM_CACHE:
        _PROGRAM_CACHE[key] = build_program(*key)
    nc = _PROGRAM_CACHE[key]
    res = run_bass_kernel_spmd(nc, in_maps, core_ids=list(range(8)))
    out = np.empty((T, B, E), np.float32)
    for core in range(8):
        b = core // 2
        th = core % 2
        out[th * TLOC:(th + 1) * TLOC, b, :] = res.results[core]["out"]
    if host_ln3 is not None:
        g3, b3g = host_ln3
        out = out * g3 + b3g
    return out



# revision 14
# speedup vs baseline: 1.6510x; 1.6510x over previous
"""Trainium2 Bass kernel for nn_DecoderBlock (T=S=1024, B=4, E=1024, H=16,
DH=64, DFF=4096) on 8 NeuronCores.

Sharding: core = b*2 + th  (b in 0..3 batches, th in {0,1} query halves).
Each core computes the full decoder block for its 512 query rows of its
batch; no collectives.

Layout/pipeline design (v2):
- All matmul weights are host-relaid so every weight DMA is a straight
  2D slice with >=1KB contiguous lines.
- Attention runs as one software-pipelined stream per head pair:
  Q(p)/K(p) projection -> scores (tile_position row-halves) -> exp on
  Scalar -> mask mult split DVE/GpSimd -> attnV lagged one key block.
  The softmax denominator is folded into attnV as a 65th lhsT column of
  ones; normalization uses a bf16 ones-matmul broadcast of z plus
  reciprocal_approx_fast on DVE.  This keeps the Tensor engine free of
  denominator/broadcast fp32 matmuls and avoids engine ping-pong (the
  p-state ramp to 2.4 GHz needs continuous PE busy).
- PSUM budget: tag "sps" bufs=4 (scores/QKV accum/broadcast/transposes)
  + tag "zt" bufs=4 (per-head attnV accumulators) = 8 banks.

Matmuls run in bf16 with fp32 PSUM accumulation; softmax/normalization/
layernorm statistics run in fp32.
"""
import sys

sys.path.insert(0, "/opt/trn_rl_repo")

import numpy as np
import ml_dtypes

import concourse.bass as bass
from concourse import bacc
import concourse.mybir as mybir
import concourse.tile as tile
from concourse.bass_utils import run_bass_kernel_spmd
from concourse.masks import make_identity

F32 = mybir.dt.float32
BF16 = mybir.dt.bfloat16
AF = mybir.ActivationFunctionType
ALU = mybir.AluOpType
BF16NP = ml_dtypes.bfloat16

T = 1024
B = 4
E = 1024
H = 16
DH = 64
DFF = 4096
P = 128
TLOC = T // 2          # query rows per core
NTB = TLOC // P        # 4 query-row blocks per core
NEC = E // P           # 8 feature chunks
NFC = DFF // P         # 32 ffn chunks

_PROGRAM_CACHE = {}


def _attention(nc, po, *, yqT, kvT, kbs, wq, wk, wv, wp, bp_sb,
               mask_T=None, resid, out_pre, tag, pre_proj=None):
    """One multi-head attention + projection + residual, emitted as a
    single software-pipelined stream.

    yqT:  SBUF [128, NEC, TLOC] bf16 — query-side activations, T-layout
    kvT:  SBUF [128, NEC, kbs*128] bf16 — key/value side, T-layout
    wq/wk/wv: DRAM [128, 8192] bf16 host-relaid (see _prep_inputs)
    wp: DRAM [E, E] bf16 row-major
    mask_T: SBUF [128, kbs, TLOC] bf16 0/1 keep-mask or None
    resid/out_pre: [128, NTB, E] f32 R-layout tiles
    """
    SKV = kbs * P
    nsh = (SKV + 511) // 512

    qT = po["cols"].tile([P, NEC, TLOC], BF16, tag="colsBF", name=f"{tag}_qT")
    kT = po["attn"].tile([P, NEC, SKV], BF16, tag="kT", bufs=1,
                         name=f"{tag}_kT")
    # V with a ones column folded in per head: [keys, kb, head, 64+1]
    vRt = po["attn"].tile([P, kbs, H, 65], BF16, tag="vRt", bufs=1,
                          name=f"{tag}_vRt")
    nc.vector.memset(vRt[:, :, :, 64:65], 1.0)
    oT = po["cols"].tile([P, NEC, TLOC], BF16, tag="colsBF", name=f"{tag}_oT")
    wp_ts = []

    def emit_q(p):
        wq_t = po["w"].tile([P, NEC * P], BF16, tag="w_q", bufs=3,
                            name=f"{tag}_wq{p}")
        nc.sync.dma_start(wq_t[:], wq[:, p * 1024:(p + 1) * 1024])
        ps = po["psA"].tile([P, 512], F32, tag="sps", name=f"{tag}_psq{p}")
        for ec in range(NEC):
            nc.tensor.matmul(ps[:, :TLOC], wq_t[:, ec * P:(ec + 1) * P],
                             yqT[:, ec, :],
                             start=(ec == 0), stop=(ec == NEC - 1))
        nc.vector.tensor_copy(qT[:, p, :], ps[:, :TLOC])

    def emit_k(p):
        wk_t = po["w"].tile([P, NEC * P], BF16, tag="w_k", bufs=3,
                            name=f"{tag}_wk{p}")
        nc.sync.dma_start(wk_t[:], wk[:, p * 1024:(p + 1) * 1024])
        for sh in range(nsh):
            w = min(512, SKV - sh * 512)
            ps = po["psA"].tile([P, 512], F32, tag="sps",
                                name=f"{tag}_psk{p}_{sh}")
            for ec in range(NEC):
                nc.tensor.matmul(ps[:, :w], wk_t[:, ec * P:(ec + 1) * P],
                                 kvT[:, ec, sh * 512: sh * 512 + w],
                                 start=(ec == 0), stop=(ec == NEC - 1))
            nc.vector.tensor_copy(kT[:, p, sh * 512: sh * 512 + w],
                                  ps[:, :w])

    def emit_v(eh):
        wv_t = po["w"].tile([P, NEC * 512], BF16, tag="w_v", bufs=2,
                            name=f"{tag}_wv{eh}")
        nc.sync.dma_start(wv_t[:], wv[:, eh * 4096:(eh + 1) * 4096])
        for sc in range(kbs):
            ps = po["psA"].tile([P, 8, 64], F32, tag="sps",
                                name=f"{tag}_psv{eh}_{sc}")
            for ec in range(NEC):
                nc.tensor.matmul(ps[:], kvT[:, ec, sc * P:(sc + 1) * P],
                                 wv_t[:, ec * 512:(ec + 1) * 512],
                                 start=(ec == 0), stop=(ec == NEC - 1))
            nc.vector.tensor_copy(vRt[:, sc, eh * 8:(eh + 1) * 8, 0:64],
                                  ps[:])

    pending_norm = [None]

    def emit_pair(p):
        h0, h1 = 2 * p, 2 * p + 1
        if kbs < 3 and pending_norm[0] is not None:
            pending_norm[0]()
            pending_norm[0] = None
        pTs = {}
        zts = {}
        for hi, h in enumerate((h0, h1)):
            pTs[h] = po["attn"].tile([P, kbs, TLOC], BF16, tag="pT",
                                     bufs=4, name=f"{tag}_pT{h}")
            zts[h] = po["psB"].tile([65, TLOC], F32, tag="zt", bufs=4,
                                    name=f"{tag}_zt{h}")
        for kb in range(kbs):
            if kb == 2 and pending_norm[0] is not None:
                pending_norm[0]()
                pending_norm[0] = None
            for hi, h in enumerate((h0, h1)):
                hb = 64 * hi
                sps = po["psA"].tile([P, 512], F32, tag="sps",
                                     name=f"{tag}_sps{h}_{kb}")
                nc.tensor.matmul(
                    sps[:, :TLOC],
                    kT[hb:hb + 64, p, kb * P:(kb + 1) * P],
                    qT[hb:hb + 64, p, :],
                    start=True, stop=(mask_T is None),
                    tile_position=(hb, 0))
                if mask_T is not None:
                    nc.tensor.matmul(sps[:, :TLOC], po["ident"][:],
                                     mask_T[:, kb, :], start=False, stop=True)
                nc.scalar.activation(pTs[h][:, kb, :], sps[:, :TLOC], AF.Exp)
            if kb >= 1:
                for h in (h0, h1):
                    nc.tensor.matmul(zts[h][0:65, :], vRt[:, kb - 1, h, :],
                                     pTs[h][:, kb - 1, :],
                                     start=(kb == 1), stop=False)
        for h in (h0, h1):
            nc.tensor.matmul(zts[h][0:65, :], vRt[:, kbs - 1, h, :],
                             pTs[h][:, kbs - 1, :],
                             start=(kbs == 1), stop=True)

        def norm(p=p, h0=h0, h1=h1, zts=zts):
            zrows = {}
            for h in (h0, h1):
                zr = po["scr"].tile([1, TLOC], BF16, tag="zrow", bufs=4,
                                    name=f"{tag}_zr{h}")
                nc.vector.tensor_copy(zr[:], zts[h][64:65, :])
                zrows[h] = zr
            bcp = po["psA"].tile([P, 512], F32, tag="sps",
                                 name=f"{tag}_bcp{p}")
            nc.tensor.matmul(bcp[0:64, :TLOC], po["ones_bf"][0:1, 0:64],
                             zrows[h0][0:1, :], start=True, stop=True,
                             tile_position=(0, 0))
            nc.tensor.matmul(bcp[64:128, :TLOC], po["ones_bf"][0:1, 0:64],
                             zrows[h1][0:1, :], start=True, stop=True,
                             tile_position=(0, 64))
            rcp = po["scr"].tile([P, TLOC], F32, tag="rcp", bufs=2,
                                 name=f"{tag}_rcp{p}")
            nc.vector.reciprocal_approx_fast(out=rcp[:], in_=bcp[:, :TLOC])
            nc.vector.tensor_tensor(oT[0:64, p, :], zts[h0][0:64, :],
                                    rcp[0:64, :], ALU.mult)
            nc.vector.tensor_tensor(oT[64:128, p, :], zts[h1][0:64, :],
                                    rcp[64:128, :], ALU.mult)
        pending_norm[0] = norm

    # ---- pipelined emission ----
    for p in range(8):
        emit_q(p)
        emit_k(p)
        if p == 0:
            emit_v(0)
        elif p == 2:
            emit_v(1)
        elif p == 4:
            for pp in range(8):
                wp_t = po["w"].tile([P, E], BF16, tag="w_p", bufs=8,
                                    name=f"{tag}_wp{pp}")
                nc.gpsimd.dma_start(wp_t[:], wp[pp * P:(pp + 1) * P, :])
                wp_ts.append(wp_t)
            if pre_proj is not None:
                pre_proj()
        emit_pair(p)
    if pending_norm[0] is not None:
        pending_norm[0]()
        pending_norm[0] = None

    # ---- projection + bias + residual ----
    for tb in range(NTB):
        for eh in range(2):
            ps = po["psA"].tile([P, 512], F32, tag="sps",
                                name=f"{tag}_pspr{tb}_{eh}")
            for p in range(8):
                nc.tensor.matmul(ps[:], oT[:, p, tb * P:(tb + 1) * P],
                                 wp_ts[p][:, eh * 512:(eh + 1) * 512],
                                 start=(p == 0), stop=False)
            nc.tensor.matmul(ps[:], po["ones_bf"][0:1, 0:P],
                             bp_sb[0:1, eh * 512:(eh + 1) * 512],
                             start=False, stop=True)
            nc.vector.tensor_tensor(
                out_pre[:, tb, eh * 512:(eh + 1) * 512], ps[:],
                resid[:, tb, eh * 512:(eh + 1) * 512], ALU.add)


def _ln_and_transpose(nc, po, *, src, outR, dst_T=None, gb=None, tag="",
                      per_tb_done=None):
    """Per-row layernorm of [128, NTB, E] f32 + optional bf16 transpose to
    T-layout [128, NEC, TLOC]."""
    for tb in range(NTB):
        stats = po["scr"].tile([P, 2, 6], F32, tag="ln_st",
                               name=f"{tag}_st{tb}")
        nc.vector.bn_stats(stats[:, 0, :], src[:, tb, 0:512])
        nc.vector.bn_stats(stats[:, 1, :], src[:, tb, 512:1024])
        mv = po["scr"].tile([P, 2], F32, tag="ln_mv", name=f"{tag}_mv{tb}")
        nc.vector.bn_aggr(mv[:], stats[:])
        rstd = po["scr"].tile([P, 1], F32, tag="ln_rs", name=f"{tag}_rs{tb}")
        nc.scalar.activation(rstd[:], mv[:, 1:2], AF.Sqrt,
                             bias=po["eps"][:])
        nc.vector.reciprocal(out=rstd[:], in_=rstd[:])
        nmrs = po["scr"].tile([P, 1], F32, tag="ln_nm", name=f"{tag}_nm{tb}")
        nc.vector.tensor_scalar(nmrs[:], mv[:, 0:1], -1.0, rstd[:],
                                ALU.mult, ALU.mult)
        if dst_T is not None:
            ybf = po["scr"].tile([P, E], BF16, tag="ybf", name=f"{tag}_yb{tb}")
            nc.scalar.activation(ybf[:], src[:, tb, :], AF.Identity,
                                 bias=nmrs[:], scale=rstd[:])
            for eg in range(2):
                pt = po["psA"].tile([P, 4, P], BF16, tag="sps",
                                    name=f"{tag}_tr{tb}_{eg}")
                for j in range(4):
                    ec = eg * 4 + j
                    nc.tensor.transpose(pt[:, j, :],
                                        ybf[:, ec * P:(ec + 1) * P],
                                        po["ident"][:])
                for j in range(4):
                    ec = eg * 4 + j
                    nc.scalar.copy(dst_T[:, ec, tb * P:(tb + 1) * P],
                                   pt[:, j, :])
        nc.scalar.activation(outR[:, tb, :], src[:, tb, :], AF.Identity,
                             bias=nmrs[:], scale=rstd[:])
        if gb is not None:
            g_bc, b_bc = gb
            nc.vector.tensor_tensor(outR[:, tb, :], outR[:, tb, :],
                                    g_bc[:], ALU.mult)
            nc.vector.tensor_tensor(outR[:, tb, :], outR[:, tb, :],
                                    b_bc[:], ALU.add)
        if per_tb_done is not None:
            per_tb_done(tb)


def _broadcast_row(nc, po, src_row, width, tag):
    """Broadcast [1, width] f32 SBUF row (base 0) -> [128, width] f32."""
    out = po["persist"].tile([P, E], F32, tag=tag, name=tag)
    for c in range(0, width, 512):
        w = min(512, width - c)
        ps = po["psA"].tile([P, 512], F32, tag="sps", name=f"{tag}_bc{c}")
        nc.tensor.matmul(ps[0:P, :w], po["ones_f32"][0:1, 0:P],
                         src_row[0:1, c:c + w], start=True, stop=True)
        nc.scalar.copy(out[:, c:c + w], ps[:, :w])
    return out


def build_program(kbs_s, kbs_c, use_mask_s, use_mask_c, use_gb):
    nc = bacc.Bacc("TRN2", target_bir_lowering=False, debug=False,
                   num_devices=8)
    SKV_S = kbs_s * P
    SKV_C = kbs_c * P

    def di(name, shape, dt=BF16):
        return nc.dram_tensor(name, shape, dt, kind="ExternalInput")

    xTq = di("xTq", [P, NEC * TLOC])
    xTkv = di("xTkv", [P, NEC * SKV_S])
    xres = di("xres", [TLOC, E], F32)
    yencT = di("yencT", [P, NEC * SKV_C])
    wq1 = di("wq1", [P, 8192]); wk1 = di("wk1", [P, 8192])
    wv1 = di("wv1", [P, 8192])
    wp1 = di("wp1", [E, E]); bp1 = di("bp1", [1, E])
    wq2 = di("wq2", [P, 8192]); wk2 = di("wk2", [P, 8192])
    wv2 = di("wv2", [P, 8192])
    wp2 = di("wp2", [E, E]); bp2 = di("bp2", [1, E])
    w1 = di("w1", [P, NFC * 1024]); b1c = di("b1c", [P, NFC], F32)
    w2 = di("w2", [P, NFC * 1024]); b2 = di("b2", [1, E])
    if use_mask_s:
        mask_s = di("mask_s", [P, kbs_s * TLOC])
    if use_mask_c:
        mask_c = di("mask_c", [P, kbs_c * TLOC])
    if use_gb:
        lngb = di("lngb", [1, 6 * E], F32)
    out = nc.dram_tensor("out", [TLOC, E], F32, kind="ExternalOutput")

    with tile.TileContext(nc) as tc:
        with (
            tc.tile_pool(name="persist", bufs=1) as persist,
            tc.tile_pool(name="rows", bufs=2) as rows,
            tc.tile_pool(name="cols", bufs=3) as cols,
            tc.tile_pool(name="wpool", bufs=2) as wpool,
            tc.tile_pool(name="scr", bufs=2) as scr,
            tc.tile_pool(name="psA", bufs=4, space="PSUM") as psA,
        ):
            po = dict(persist=persist, rows=rows, cols=cols, w=wpool,
                      scr=scr, psA=psA)

            ones_bf = persist.tile([P, P], BF16)
            nc.vector.memset(ones_bf[:], 1.0)
            ones_f32 = persist.tile([P, 64], F32)
            nc.vector.memset(ones_f32[:], 1.0)
            ident = persist.tile([P, P], BF16)
            make_identity(nc, ident[:])
            eps_tile = persist.tile([P, 1], F32)
            nc.vector.memset(eps_tile[:], 1e-5)
            po.update(ones_bf=ones_bf, ones_f32=ones_f32, ident=ident,
                      eps=eps_tile)

            # query-side activations first so Q matmuls can start early
            xTq_sb = cols.tile([P, NEC, TLOC], BF16, tag="colsBF",
                               name="xTqS")
            nc.sync.dma_start(
                xTq_sb[:], xTq.rearrange("p (eo t) -> p eo t", eo=NEC))

            with (
                tc.tile_pool(name="attn_sb", bufs=2) as attn_sb,
                tc.tile_pool(name="psB", bufs=4, space="PSUM") as psB,
            ):
                po["attn"] = attn_sb
                po["psB"] = psB

                xTkv_sb = attn_sb.tile([P, NEC, SKV_S], BF16, tag="kvT_s",
                                       bufs=1, name="xTkvS")
                nc.gpsimd.dma_start(
                    xTkv_sb[:],
                    xTkv.rearrange("p (eo s) -> p eo s", eo=NEC))
                mask_s_sb = None
                if use_mask_s:
                    mask_s_sb = attn_sb.tile([P, kbs_s, TLOC], BF16,
                                             tag="mask_s", bufs=1,
                                             name="mask_sS")
                    nc.gpsimd.dma_start(
                        mask_s_sb[:],
                        mask_s.rearrange("p (kb t) -> p kb t", kb=kbs_s))
                bp1_sb = persist.tile([1, E], BF16, tag="bp1", name="bp1s")
                nc.sync.dma_start(bp1_sb[:], bp1[:])

                xres_sb = rows.tile([P, NTB, E], F32, tag="rowsF32",
                                    name="xresS")
                yencT_sb = attn_sb.tile([P, NEC, SKV_C], BF16, tag="kvT_c",
                                        bufs=1, name="yencTS")
                mask_c_sb = None
                if use_mask_c:
                    mask_c_sb = attn_sb.tile([P, kbs_c, TLOC], BF16,
                                             tag="mask_c", bufs=1,
                                             name="mask_cS")

                def fill_xres():
                    for tb in range(NTB):
                        nc.gpsimd.dma_start(xres_sb[:, tb, :],
                                            xres[tb * P:(tb + 1) * P, :])
                    nc.gpsimd.dma_start(
                        yencT_sb[:],
                        yencT.rearrange("p (eo s) -> p eo s", eo=NEC))
                    if mask_c_sb is not None:
                        nc.gpsimd.dma_start(
                            mask_c_sb[:],
                            mask_c.rearrange("p (kb t) -> p kb t", kb=kbs_c))

                y1pre = rows.tile([P, NTB, E], F32, tag="rowsF32",
                                  name="y1pre")
                y1R = rows.tile([P, NTB, E], F32, tag="rowsF32", name="y1R")
                y1T = cols.tile([P, NEC, TLOC], BF16, tag="colsBF",
                                name="y1T")
                y2pre = rows.tile([P, NTB, E], F32, tag="rowsF32",
                                  name="y2pre")
                y2R = rows.tile([P, NTB, E], F32, tag="rowsF32", name="y2R")
                y2T = cols.tile([P, NEC, TLOC], BF16, tag="colsBF",
                                name="y2T")

                _attention(nc, po, yqT=xTq_sb, kvT=xTkv_sb, kbs=kbs_s,
                           wq=wq1, wk=wk1, wv=wv1, wp=wp1, bp_sb=bp1_sb,
                           mask_T=mask_s_sb, resid=xres_sb, out_pre=y1pre,
                           tag="sa", pre_proj=fill_xres)

                gbs = [None, None, None]
                if use_gb:
                    gbrow = persist.tile([1, 6 * E], F32, tag="lngb",
                                         name="gbr")
                    nc.sync.dma_start(gbrow[:], lngb[:])
                    for i in range(3):
                        g_bc = _broadcast_row(
                            nc, po, gbrow[:, 2 * i * E:(2 * i + 1) * E], E,
                            f"g_bc{i}")
                        b_bc = _broadcast_row(
                            nc, po, gbrow[:, (2 * i + 1) * E:(2 * i + 2) * E],
                            E, f"b_bc{i}")
                        gbs[i] = (g_bc, b_bc)

                _ln_and_transpose(nc, po, src=y1pre, outR=y1R, dst_T=y1T,
                                  gb=gbs[0], tag="ln1")

                bp2_sb = persist.tile([1, E], BF16, tag="bp2", name="bp2s")
                nc.sync.dma_start(bp2_sb[:], bp2[:])
                _attention(nc, po, yqT=y1T, kvT=yencT_sb, kbs=kbs_c,
                           wq=wq2, wk=wk2, wv=wv2, wp=wp2, bp_sb=bp2_sb,
                           mask_T=mask_c_sb, resid=y1R, out_pre=y2pre,
                           tag="ca")
                _ln_and_transpose(nc, po, src=y2pre, outR=y2R, dst_T=y2T,
                                  gb=gbs[1], tag="ln2")

            with (
                tc.tile_pool(name="ffn_sb", bufs=1) as ffn_sb,
                tc.tile_pool(name="psC", bufs=4, space="PSUM") as psC,
            ):
                b1_sb = persist.tile([P, NFC], F32, tag="b1c", name="b1s")
                nc.sync.dma_start(b1_sb[:], b1c[:])
                b2_sb = persist.tile([1, E], BF16, tag="b2", name="b2s")
                nc.sync.dma_start(b2_sb[:], b2[:])

                hT = ffn_sb.tile([P, NFC, TLOC], BF16, tag="hT", name="hT")
                for fcp in range(NFC // 2):
                    w1_t = ffn_sb.tile([P, 2, NEC * P], BF16, tag="w_f1",
                                       bufs=3, name=f"w1_{fcp}")
                    nc.sync.dma_start(
                        w1_t[:],
                        w1[:, fcp * 2048:(fcp + 1) * 2048].rearrange(
                            "p (f c) -> p f c", f=2))
                    for fi in range(2):
                        fc = fcp * 2 + fi
                        ps = psA.tile([P, 512], F32, tag="sps",
                                      name=f"psf1_{fc}")
                        for ec in range(NEC):
                            nc.tensor.matmul(
                                ps[:, :TLOC],
                                w1_t[:, fi, ec * P:(ec + 1) * P],
                                y2T[:, ec, :],
                                start=(ec == 0), stop=(ec == NEC - 1))
                        nc.scalar.activation(hT[:, fc, :], ps[:, :TLOC],
                                             AF.Relu,
                                             bias=b1_sb[:, fc:fc + 1])

                y3pre = rows.tile([P, NTB, E], F32, tag="rowsF32",
                                  name="y3pre")
                for eh in range(2):
                    pss = [psC.tile([P, 512], F32, tag="ps_f2",
                                    name=f"psf2_{eh}_{tb}")
                           for tb in range(NTB)]
                    for fcp in range(NFC // 2):
                        w2_t = ffn_sb.tile([P, 2, E], BF16, tag="w_f2",
                                           bufs=4, name=f"w2_{eh}_{fcp}")
                        nc.sync.dma_start(
                            w2_t[:],
                            w2[:, fcp * 2048:(fcp + 1) * 2048].rearrange(
                                "p (f c) -> p f c", f=2))
                        for fi in range(2):
                            fc = fcp * 2 + fi
                            for tb in range(NTB):
                                nc.tensor.matmul(
                                    pss[tb][:],
                                    hT[:, fc, tb * P:(tb + 1) * P],
                                    w2_t[:, fi, eh * 512:(eh + 1) * 512],
                                    start=(fc == 0), stop=False)
                    for tb in range(NTB):
                        nc.tensor.matmul(pss[tb][:], ones_bf[0:1, 0:P],
                                         b2_sb[0:1, eh * 512:(eh + 1) * 512],
                                         start=False, stop=True)
                        nc.vector.tensor_tensor(
                            y3pre[:, tb, eh * 512:(eh + 1) * 512],
                            pss[tb][:],
                            y2R[:, tb, eh * 512:(eh + 1) * 512], ALU.add)

                outR = rows.tile([P, NTB, E], F32, tag="rowsF32",
                                 name="outR")

                def ship_out(tb):
                    nc.sync.dma_start(out[tb * P:(tb + 1) * P, :],
                                      outR[:, tb, :])

                _ln_and_transpose(nc, po, src=y3pre, outR=outR, gb=gbs[2],
                                  tag="ln3", per_tb_done=ship_out)

    nc.compile()
    return nc


def _prep_inputs(inputs):
    """Host-side prep: returns (program_key, 8 in_maps, host_ln3)."""
    tgt = np.asarray(inputs["tgt"], np.float32)
    yenc = np.asarray(inputs["Y_enc_out"], np.float32)
    tgt_mask = np.asarray(inputs["tgt_mask"], np.float32)
    spm = np.asarray(inputs["src_padding_mask"])
    tpm = np.asarray(inputs["tgt_padding_mask"])

    causal = np.isneginf(tgt_mask) | np.isnan(tgt_mask)   # [Tq, Sk]
    masked_s = causal[None, :, :] | tpm[:, None, :]       # [B, Tq, Sk]
    masked_c = np.zeros((B, T, T), bool) | spm[:, None, :]

    live_s = ~masked_s.all(axis=(0, 1))
    live_c = ~masked_c.all(axis=(0, 1))
    kbs_s = max(1, -(-int(np.max(np.nonzero(live_s)[0], initial=0) + 1) // P))
    kbs_c = max(1, -(-int(np.max(np.nonzero(live_c)[0], initial=0) + 1) // P))

    keep_s = (~masked_s[:, :, :kbs_s * P]).astype(np.float32)
    keep_c = (~masked_c[:, :, :kbs_c * P]).astype(np.float32)
    mb_s = (-30.0 * (1.0 - keep_s)).astype(np.float32)
    mb_c = (-30.0 * (1.0 - keep_c)).astype(np.float32)
    use_mask_s = not np.all(keep_s == 1.0)
    use_mask_c = not np.all(keep_c == 1.0)

    g1 = np.asarray(inputs["ln1_g"], np.float32)
    b1g = np.asarray(inputs["ln1_b"], np.float32)
    g2 = np.asarray(inputs["ln2_g"], np.float32)
    b2g = np.asarray(inputs["ln2_b"], np.float32)
    g3 = np.asarray(inputs["ln3_g"], np.float32)
    b3g = np.asarray(inputs["ln3_b"], np.float32)
    use_gb = not (np.all(g1 == 1) and np.all(g2 == 1) and np.all(b1g == 0)
                  and np.all(b2g == 0))
    host_ln3 = None
    if not (np.all(g3 == 1) and np.all(b3g == 0)):
        host_ln3 = (g3, b3g)

    def heads_cols(w):  # [H, E, DH] -> [E, H*DH]
        return np.ascontiguousarray(
            np.asarray(w, np.float32).transpose(1, 0, 2).reshape(E, E))

    def qk_layout(hc):  # [E, E] -> [128, 8*8*128]: slice p -> [pp, eo, c]
        return np.ascontiguousarray(
            hc.reshape(8, 128, 8, 128).transpose(1, 2, 0, 3).reshape(128,
                                                                     8192))

    def v_layout(hc):  # [E, E] -> [128, 2*8*512]: slice eh -> [pp, eo, c]
        return np.ascontiguousarray(
            hc.reshape(8, 128, 2, 512).transpose(1, 2, 0, 3).reshape(128,
                                                                     8192))

    scale = 1.0 / np.sqrt(np.float32(DH))
    wq1 = qk_layout(heads_cols(inputs["Wq1"]) * scale).astype(BF16NP)
    wk1 = qk_layout(heads_cols(inputs["Wk1"])).astype(BF16NP)
    wv1 = v_layout(heads_cols(inputs["Wv1"])).astype(BF16NP)
    wq2 = qk_layout(heads_cols(inputs["Wq2"]) * scale).astype(BF16NP)
    wk2 = qk_layout(heads_cols(inputs["Wk2"])).astype(BF16NP)
    wv2 = v_layout(heads_cols(inputs["Wv2"])).astype(BF16NP)
    wp1 = np.asarray(inputs["Wp1"], np.float32).astype(BF16NP)
    wp2 = np.asarray(inputs["Wp2"], np.float32).astype(BF16NP)
    # w1: [E, DFF] -> [128, 32*8*128]: slice fc -> [pp, eo, c]
    w1 = np.ascontiguousarray(
        np.asarray(inputs["W1"], np.float32).reshape(8, 128, 32, 128)
        .transpose(1, 2, 0, 3).reshape(128, NFC * 1024)).astype(BF16NP)
    # w2: [DFF, E] -> [128, 32*1024]: slice fc -> [pp, e]
    w2 = np.ascontiguousarray(
        np.asarray(inputs["W2"], np.float32).reshape(32, 128, 1024)
        .transpose(1, 0, 2).reshape(128, NFC * 1024)).astype(BF16NP)
    bp1 = np.asarray(inputs["bp1"], np.float32).reshape(1, E).astype(BF16NP)
    bp2 = np.asarray(inputs["bp2"], np.float32).reshape(1, E).astype(BF16NP)
    b2v = np.asarray(inputs["b2"], np.float32).reshape(1, E).astype(BF16NP)
    b1c = np.ascontiguousarray(
        np.asarray(inputs["b1"], np.float32).reshape(NFC, P).T)
    lngb = np.concatenate([g1, b1g, g2, b2g, g3, b3g]).reshape(1, 6 * E)

    def t_layout(xT, cols_):  # [E, cols] -> [128, 8*cols]
        return np.ascontiguousarray(
            xT.reshape(8, 128, cols_).transpose(1, 0, 2).reshape(
                128, 8 * cols_))

    def mask_layout(keepT):  # [SKV, TLOC] -> [128, kb*TLOC]
        kb = keepT.shape[0] // P
        return np.ascontiguousarray(
            keepT.reshape(kb, 128, TLOC).transpose(1, 0, 2).reshape(
                128, kb * TLOC))

    in_maps = []
    for core in range(8):
        b = core // 2
        th = core % 2
        t0 = th * TLOC
        xb = tgt[:, b, :]
        xT = np.ascontiguousarray(xb.T)
        m = {
            "xTq": t_layout(
                np.ascontiguousarray(xT[:, t0:t0 + TLOC]), TLOC
            ).astype(BF16NP),
            "xTkv": t_layout(
                np.ascontiguousarray(xT[:, :kbs_s * P]), kbs_s * P
            ).astype(BF16NP),
            "xres": np.ascontiguousarray(xb[t0:t0 + TLOC, :]),
            "yencT": t_layout(
                np.ascontiguousarray(yenc[:kbs_c * P, b, :].T), kbs_c * P
            ).astype(BF16NP),
            "wq1": wq1, "wk1": wk1, "wv1": wv1, "wp1": wp1, "bp1": bp1,
            "wq2": wq2, "wk2": wk2, "wv2": wv2, "wp2": wp2, "bp2": bp2,
            "w1": w1, "b1c": b1c, "w2": w2, "b2": b2v,
        }
        if use_mask_s:
            m["mask_s"] = mask_layout(
                np.ascontiguousarray(
                    mb_s[b, t0:t0 + TLOC, :].T)).astype(BF16NP)
        if use_mask_c:
            m["mask_c"] = mask_layout(
                np.ascontiguousarray(
                    mb_c[b, t0:t0 + TLOC, :].T)).astype(BF16NP)
        if use_gb:
            m["lngb"] = lngb
        in_maps.append(m)

    key = (kbs_s, kbs_c, use_mask_s, use_mask_c, use_gb)
    return key, in_maps, host_ln3


def kernel(**inputs) -> np.ndarray:
    key, in_maps, host_ln3 = _prep_inputs(inputs)
    if key not in _PROGRAM_CACHE:
        _PROGRAM_CACHE[key] = build_program(*key)
    nc = _PROGRAM_CACHE[key]
    res = run_bass_kernel_spmd(nc, in_maps, core_ids=list(range(8)))
    out = np.empty((T, B, E), np.float32)
    for core in range(8):
        b = core // 2
        th = core % 2
        out[th * TLOC:(th + 1) * TLOC, b, :] = res.results[core]["out"]
    if host_ln3 is not None:
        g3, b3g = host_ln3
        out = out * g3 + b3g
    return out
